# revision 20
# baseline (speedup 1.0000x reference)
"""BFP (block-floating-point) quantized linear on 8 TRN2 NeuronCores. v6

out = quantize_bfp(x) @ quantize_bfp(weight).T + bias
  - groups of 32 contiguous elements along the contraction dim share one
    exponent e = floor(log2(max_abs)); scale s = 2^(e-7);
    q = clip(round(v/s), -128, 127) * s  (round half-to-even), zero-guarded.

Layout: PM=2 x PN=4 grid; each core computes a [4096, 1024] output block
with K=4096, W held resident in SBUF (transposed, bf16), x streamed.

Key design points (727us baseline -> ~610us):
  * host casts x/weight to bf16 with round-toward-zero (mantissa
    truncation): halves load DMA and SBUF staging, and RTZ provably
    preserves floor(log2(group_max)) so the BFP exponent always matches
    the f32 reference (RNE could bump gmax across a power of two and
    change which elements the reference's clip-at-127 slashes).
  * quantize on DVE per half-tile: abs-max group reduce (bf16), exponent
    via int16 bitfield ops, fused round+clip custom DVE op against
    C = 1.5*2^23*s (magic-number rounding, exact in f32 internals).
  * input loads ride the ACT HWDGE queue so the sync queue only carries
    xbar transposes (no head-of-line blocking of prefetches).
  * no bias matmuls on the PE: bias broadcast [128, n_sh] once via a K=1
    outer-product matmul, added during drain on the DVE at 2x; drains
    (ACT copy + DVE add + SWDGE store) trail the matmuls by two strips.
  * startup: first strip piece is 256 wide and split at K=2048 so the PE
    starts as soon as w0,w1 + x0 half-tiles are quantized+transposed;
    W-phase emits 256-wide pieces gated on w-tile pairs.
"""

import numpy as np
import ml_dtypes

import concourse.bass as bass
import concourse.tile as tile
from concourse import bacc, mybir
from concourse._compat import with_exitstack
from concourse.bass_utils import run_bass_kernel_spmd

DT = mybir.dt

M, IN, OUT = 8192, 4096, 4096
PM, PN = 2, 4
M_SH, N_SH = M // PM, OUT // PN  # 4096, 1024 per core
GS = 32          # bfp group size
P = 128          # partitions
NT = 512         # matmul strip width (one PSUM bank of f32)
KC = IN // P     # 32 k-chunks

_C_MUL = 98304.0              # 1.5 * 2^16: C = 2^e * _C_MUL = 1.5*2^23*s
_HI_K = 127.0 / 12582912.0    # C * _HI_K = 127 * s
_LO_K = -1.0 / 98304.0        # C * _LO_K = -128 * s
_EXP_MASK16 = 0x7F80          # bf16 exponent field
_EXP_MIN16 = 0x0080           # clamp exponent field >= 1 (zero-group guard)

# ---------------------------------------------------------------------------
# custom fused DVE op: out = clip(round_to_multiple(x, s), -128s, 127s)
# ---------------------------------------------------------------------------
_BFP_OP = None


def _bfp_apply_ref(in0, in1, c0, c1, c2):
    x = np.asarray(in0, np.float32)
    C = np.asarray(in1, np.float32).reshape(x.shape)
    t = ((x + C).astype(np.float32) - C).astype(np.float32)
    hi = (C * np.float32(c0)).astype(np.float32)
    lo = (C * np.float32(c1)).astype(np.float32)
    return np.maximum(np.minimum(t, hi), lo)


def get_bfp_op():
    global _BFP_OP
    if _BFP_OP is not None:
        return _BFP_OP
    from concourse.dve_ops import (
        CUSTOM_DVE_SPECS,
        OPS,
        _CUSTOM_DVE_ROW_BASE,
        _SUB_OPCODE_FOR_NAME,
        DveOp,
    )
    from concourse.dve_spec import C0, C1, Spec, Src0, Src1, lower, maxx, minn
    from concourse.dve_uop import DveOpSpec

    for existing in OPS:
        if existing.name == "BFP_APPLY_ANT":
            _BFP_OP = existing
            return existing

    t = (Src0 + Src1) - Src1
    spec = Spec(
        body=maxx(minn(t, Src1 * C0), Src1 * C1),
        reference=_bfp_apply_ref,
    )
    shas = {
        ver: DveOpSpec(
            name="BFP_APPLY_ANT", uops=lower(spec, ver=ver), rd1_en=True
        ).sha(ver)
        for ver in ("v3", "v4")
    }
    op = DveOp("BFP_APPLY_ANT", spec, subdim=False, uops_sha=shas)
    OPS.append(op)
    CUSTOM_DVE_SPECS[op.name] = op.spec
    _SUB_OPCODE_FOR_NAME[op.name] = _CUSTOM_DVE_ROW_BASE + len(OPS) - 1
    _BFP_OP = op
    return op


# ---------------------------------------------------------------------------
# Tile kernel builder
# ---------------------------------------------------------------------------
@with_exitstack
def build_bfl(ctx, tc, out_ap, x_ap, w_ap, b_ap, m_sh, n_sh, k):
    nc = tc.nc
    op = get_bfp_op()
    G = k // GS        # groups per row (128)
    kc = k // P        # k-chunks (32)
    n_wt = n_sh // P   # weight row-tiles (8)
    n_mt = m_sh // P   # x row-tiles (32)
    n_nt = n_sh // NT  # 512-wide strips (2)
    wt_per_nt = NT // P  # w tiles per strip (4)
    half = k // 2
    hc = half // P     # chunks per half (16)
    hg = G // 2        # groups per half (64)

    stage = ctx.enter_context(tc.tile_pool(name="stage", bufs=4))
    qpool = ctx.enter_context(tc.tile_pool(name="q", bufs=4))
    qtpool = ctx.enter_context(tc.tile_pool(name="qt", bufs=6))
    gpool = ctx.enter_context(tc.tile_pool(name="g", bufs=3))
    wqt_pool = ctx.enter_context(tc.tile_pool(name="wqt", bufs=1))
    cpool = ctx.enter_context(tc.tile_pool(name="const", bufs=1))
    opool = ctx.enter_context(tc.tile_pool(name="o", bufs=5))
    pspool = ctx.enter_context(tc.tile_pool(name="ps", bufs=5, space="PSUM"))
    psh_pool = ctx.enter_context(tc.tile_pool(name="psh", bufs=2, space="PSUM"))
    psd_pool = ctx.enter_context(tc.tile_pool(name="psd", bufs=1, space="PSUM"))

    # ---- bias broadcast [P, n_sh] via K=1 outer-product matmul; the
    # instructions are emitted mid-startup (first drain needs biasb much
    # later) so they stay off the w0/x0 critical path ----
    ones_t = cpool.tile([1, P], DT.bfloat16, tag="ones")
    bias_f = cpool.tile([1, n_sh], DT.float32, tag="bias_f")
    bias_r = cpool.tile([1, n_sh], DT.bfloat16, tag="bias_r")
    biasb = cpool.tile([P, n_sh], DT.bfloat16, tag="biasb")

    dummy_t = cpool.tile([1, NT], DT.bfloat16, tag="dummy")
    nc.vector.memset(dummy_t[:], 1.0)
    psd = psd_pool.tile([P, NT], DT.float32, tag="psd")

    def warm(n):
        """Keep-warm filler: self-contained K=1 matmuls into a scratch PSUM
        bank. Emitted at W-phase wait points so the PE's HAM activity
        window never sees a full idle window (stays at the 2.4 GHz clock);
        they execute only when the PE would otherwise be waiting."""
        for _ in range(n):
            nc.tensor.matmul(
                psd[:],
                lhsT=dummy_t[0:1, 0:P],
                rhs=dummy_t[:],
                start=True,
                stop=True,
            )

    def setup_bias():
        nc.vector.memset(ones_t[:], 1.0)
        nc.sync.dma_start(bias_f[:], b_ap.unsqueeze(0))
        nc.vector.tensor_copy(bias_r[:], bias_f[:])
        for j in range(n_sh // NT):
            psb = pspool.tile([P, NT], DT.float32, tag="ps512", name=f"psb{j}")
            nc.tensor.matmul(
                psb[:],
                lhsT=ones_t[:],
                rhs=bias_r[:, j * NT:(j + 1) * NT],
                start=True,
                stop=True,
            )
            nc.vector.tensor_copy(biasb[:, j * NT:(j + 1) * NT], psb[:])

    # ---- W quantize: tile wt -> wqt[nt][:, :, col:col+P] ----
    wqt = [
        wqt_pool.tile([P, kc * NT], DT.bfloat16, tag=f"wqt{i}", name=f"wqt{i}")
        for i in range(n_nt)
    ]

    def quantize_tile(src_dram, name, split=False):
        """bf16 rows [P, k] -> bf16 quantized [P, k]; 16-bit DVE ops.
        split=True runs the exponent ops per half (lower latency for the
        startup tiles); otherwise one fused pass over all groups."""
        xb = stage.tile([P, k], DT.bfloat16, tag="stage", name=f"st_{name}")
        nc.scalar.dma_start(xb[:], src_dram)
        gm = gpool.tile([P, G], DT.bfloat16, tag="gmax", name=f"gm_{name}")
        ci = gpool.tile([P, G], DT.int16, tag="ci", name=f"ci_{name}")
        cf = gpool.tile([P, G], DT.bfloat16, tag="cf", name=f"cf_{name}")
        q = qpool.tile([P, k], DT.bfloat16, tag="q", name=f"q_{name}")

        def reduce_half(h):
            k0, g0 = h * half, h * hg
            nc.vector.tensor_reduce(
                gm[:, g0:g0 + hg],
                xb[:, k0:k0 + half].rearrange("p (g j) -> p g j", j=GS),
                axis=mybir.AxisListType.X,
                op=mybir.AluOpType.max,
                apply_absolute_value=True,
            )

        def exp_ops(g0, g1):
            nc.vector.tensor_scalar(
                ci[:, g0:g1],
                gm[:, g0:g1].bitcast(DT.int16),
                _EXP_MASK16,
                None,
                op0=mybir.AluOpType.bitwise_and,
            )
            nc.vector.tensor_scalar_max(ci[:, g0:g1], ci[:, g0:g1], _EXP_MIN16)
            nc.vector.tensor_scalar_mul(
                cf[:, g0:g1], ci[:, g0:g1].bitcast(DT.bfloat16), _C_MUL
            )

        def apply_half(h):
            k0, g0 = h * half, h * hg
            nc.vector._custom_dve(
                op,
                out=q[:, k0:k0 + half],
                in0=xb[:, k0:k0 + half],
                in1=cf[:, g0:g0 + hg].unsqueeze(2).broadcast_to([P, hg, GS]),
                s0=_HI_K,
                s1=_LO_K,
            )

        if split:
            for h in range(2):
                reduce_half(h)
                exp_ops(h * hg, (h + 1) * hg)
                apply_half(h)
        else:
            reduce_half(0)
            reduce_half(1)
            exp_ops(0, G)
            apply_half(0)
            apply_half(1)
        return q

    def w_tile(wt, split=False):
        rows = slice(wt * P, (wt + 1) * P)
        nt, col = wt // wt_per_nt, (wt % wt_per_nt) * P
        w3 = wqt[nt][:].rearrange("p (c n) -> p c n", n=NT)
        q = quantize_tile(w_ap[rows, :], f"w{wt}", split=split)
        for h in range(2):
            nc.sync.dma_start_transpose(
                w3[:, h * hc:(h + 1) * hc, col:col + P],
                q[:, h * half:(h + 1) * half],
            )

    def x_tile(mt, split=False):
        rows = slice(mt * P, (mt + 1) * P)
        xqt = qtpool.tile([P, kc * P], DT.bfloat16, tag="xqt", name=f"xqt{mt}")
        xqt3 = xqt[:].rearrange("p (c m) -> p c m", m=P)
        q = quantize_tile(x_ap[rows, :], f"x{mt}", split=split)
        for h in range(2):
            nc.sync.dma_start_transpose(
                xqt3[:, h * hc:(h + 1) * hc, :],
                q[:, h * half:(h + 1) * half],
            )
        return xqt3

    # ---- strip pieces: accumulating matmuls + (deferred) drain ----
    # drain queue entries: (ps, mt, ncol0, c_w)
    drainq = []

    def flush_drains(keep=0):
        while len(drainq) > keep:
            ps, mt, ncol0, c_w = drainq.pop(0)
            ob = opool.tile([P, c_w], DT.bfloat16, tag=f"o{c_w}",
                            name=f"ob{mt}_{ncol0}")
            nc.scalar.copy(ob[:], ps)
            ob2 = opool.tile([P, c_w], DT.bfloat16, tag=f"p{c_w}",
                             name=f"o2{mt}_{ncol0}")
            nc.vector.tensor_tensor(
                ob2[:], ob[:], biasb[:, ncol0:ncol0 + c_w],
                op=mybir.AluOpType.add,
            )
            nc.gpsimd.dma_start(
                out_ap[mt * P:(mt + 1) * P, ncol0:ncol0 + c_w], ob2[:]
            )

    open_ps = {}

    def do_piece(mt, xqt3, c_lo, c_w, k0=0, k1=None, pool=None):
        """cols [c_lo, c_lo+c_w), k-chunks [k0, k1) of m-tile mt."""
        k1 = kc if k1 is None else k1
        key = (mt, c_lo, c_w)
        if key in open_ps:
            ps = open_ps.pop(key)
        else:
            if c_w == NT:
                psf = pspool.tile([P, NT], DT.float32, tag="ps512",
                                  name=f"ps{mt}_{c_lo}")
            else:
                psf = psh_pool.tile([P, 256], DT.float32, tag="ps256",
                                    name=f"ps{mt}_{c_lo}")
            ps = psf[:, 0:c_w]
        nt = c_lo // NT
        s_lo = c_lo - nt * NT
        wq3 = wqt[nt][:].rearrange("p (c n) -> p c n", n=NT)
        for c in range(k0, k1):
            nc.tensor.matmul(
                ps,
                lhsT=xqt3[:, c, :],
                rhs=wq3[:, c, s_lo:s_lo + c_w],
                start=(c == 0),
                stop=(c == kc - 1),
            )
        if k1 < kc:
            open_ps[key] = ps
        else:
            drainq.append((ps, mt, c_lo, c_w))

    # ---- emission order ----
    if n_mt < 8 or n_wt != 8 or n_nt != 2:
        # generic order (small shapes / simulator testing)
        for wt in range(n_wt):
            w_tile(wt)
        for mt in range(n_mt):
            xqt3 = x_tile(mt)
            for nt in range(n_nt):
                do_piece(mt, xqt3, nt * NT, NT)
            flush_drains(keep=2)
        flush_drains()
        return

    xq = {}
    # startup: w0 + x0 first; the first piece is 128 wide (w0 only) and
    # split at K=2048 so the PE starts right after the first half-tiles.
    w_tile(0, split=True)
    xq[0] = x_tile(0, split=True)
    do_piece(0, xq[0], 0, 128, k0=0, k1=hc)
    warm(20)
    do_piece(0, xq[0], 0, 128, k0=hc)
    w_tile(1, split=True)
    warm(20)
    do_piece(0, xq[0], 128, 128)
    setup_bias()
    w_tile(2)
    w_tile(3)
    warm(20)
    do_piece(0, xq[0], 256, 256)
    xq[1] = x_tile(1)
    warm(20)
    do_piece(1, xq[1], 0, 256)
    do_piece(1, xq[1], 256, 256)
    w_tile(4)
    w_tile(5)
    xq[2] = x_tile(2)
    warm(20)
    do_piece(0, xq[0], 512, 256)
    do_piece(2, xq[2], 0, 256)
    warm(20)
    do_piece(1, xq[1], 512, 256)
    do_piece(2, xq[2], 256, 256)
    w_tile(6)
    w_tile(7)
    flush_drains(keep=2)
    xq[3] = x_tile(3)
    warm(20)
    do_piece(0, xq[0], 768, 256)
    do_piece(3, xq[3], 0, 512)
    warm(20)
    do_piece(1, xq[1], 768, 256)
    do_piece(2, xq[2], 512, 256)
    xq[4] = x_tile(4)
    warm(20)
    do_piece(4, xq[4], 0, 512)
    do_piece(2, xq[2], 768, 256)
    warm(20)
    do_piece(3, xq[3], 512, 512)
    flush_drains(keep=2)
    # steady state: strip-1 lags strip-0 by one m-tile
    for mt in range(5, n_mt):
        xq[mt] = x_tile(mt)
        if mt < 11:
            warm(12)
        do_piece(mt, xq[mt], 0, 512)
        do_piece(mt - 1, xq[mt - 1], 512, 512)
        flush_drains(keep=2)
    do_piece(n_mt - 1, xq[n_mt - 1], 512, 512)
    flush_drains()


# ---------------------------------------------------------------------------
# host entry
# ---------------------------------------------------------------------------
_CACHE = {}
LAST_EXEC_NS = None
LAST_RESULTS = None


def _build(m_sh, n_sh, k, num_devices=8):
    key = (m_sh, n_sh, k)
    if key in _CACHE:
        return _CACHE[key]
    nc = bacc.Bacc(
        "TRN2",
        target_bir_lowering=False,
        debug=False,
        enable_asserts=True,
        num_devices=num_devices,
    )
    x_ap = nc.dram_tensor("x", [m_sh, k], DT.bfloat16, kind="ExternalInput").ap()
    w_ap = nc.dram_tensor("w", [n_sh, k], DT.bfloat16, kind="ExternalInput").ap()
    b_ap = nc.dram_tensor("b", [n_sh], DT.float32, kind="ExternalInput").ap()
    out_ap = nc.dram_tensor(
        "out", [m_sh, n_sh], DT.bfloat16, kind="ExternalOutput"
    ).ap()
    with tile.TileContext(nc) as tc:
        build_bfl(tc, out_ap, x_ap, w_ap, b_ap, m_sh, n_sh, k)
    nc.compile()
    _CACHE[key] = nc
    return nc


def _install_ntff_hook():
    import sys
    import types

    if "antenv.axon_hooks" in sys.modules:
        return
    try:
        from trn_agent_boot.trn_boot import _ntff_profile_via_ctypes

        hook = _ntff_profile_via_ctypes("/opt/axon/libaxon_pjrt.so")
    except Exception:
        hook = None
    mod = types.ModuleType("antenv.axon_hooks")
    state = {"hook": hook}
    mod.get_axon_ntff_profile_hook = lambda: state["hook"]
    mod.set_axon_ntff_profile_hook = lambda h: state.update(hook=h)
    sys.modules["antenv.axon_hooks"] = mod


def kernel(x, weight, bias, trace=False):
    global LAST_EXEC_NS, LAST_RESULTS
    if trace:
        _install_ntff_hook()
    # round-toward-zero f32->bf16 (mantissa truncation): preserves
    # floor(log2(group_max)) exactly, so the BFP exponent matches the
    # f32 reference's (RNE can bump gmax across a power of two, which
    # changes which elements the reference's clip-at-127 slashes).
    x = np.ascontiguousarray(np.asarray(x, np.float32))
    weight = np.ascontiguousarray(np.asarray(weight, np.float32))
    x = (x.view(np.uint32) >> 16).astype(np.uint16).view(ml_dtypes.bfloat16)
    weight = (
        (weight.view(np.uint32) >> 16).astype(np.uint16).view(ml_dtypes.bfloat16)
    )
    bias = np.ascontiguousarray(np.asarray(bias, np.float32))
    assert x.shape == (M, IN) and weight.shape == (OUT, IN) and bias.shape == (OUT,)

    nc = _build(M_SH, N_SH, IN)
    in_maps = []
    for c in range(8):
        mb, nb = c // PN, c % PN
        in_maps.append(
            {
                "x": np.ascontiguousarray(x[mb * M_SH:(mb + 1) * M_SH]),
                "w": np.ascontiguousarray(weight[nb * N_SH:(nb + 1) * N_SH]),
                "b": np.ascontiguousarray(bias[nb * N_SH:(nb + 1) * N_SH]),
            }
        )
    res = run_bass_kernel_spmd(nc, in_maps, core_ids=list(range(8)), trace=trace)
    LAST_EXEC_NS = res.exec_time_ns
    LAST_RESULTS = res
    out = np.empty((M, OUT), np.float32)
    for c in range(8):
        mb, nb = c // PN, c % PN
        out[mb * M_SH:(mb + 1) * M_SH, nb * N_SH:(nb + 1) * N_SH] = np.asarray(
            res.results[c]["out"]
        ).astype(np.float32)
    return out


# revision 21
# speedup vs baseline: 1.1632x; 1.1632x over previous
"""BFP (block-floating-point) quantized linear on 8 TRN2 NeuronCores. v5

out = quantize_bfp(x) @ quantize_bfp(weight).T + bias
  - groups of 32 contiguous elements along the contraction dim share one
    exponent e = floor(log2(max_abs)); scale s = 2^(e-7);
    q = clip(round(v/s), -128, 127) * s  (round half-to-even), zero-guarded.

v5 changes vs v4 (656us):
  * no bias matmuls on the PE: bias broadcast to [128, n_sh] once (K=1
    outer-product matmul + DVE copy), added to each drained strip on the
    DVE at 2x (emission delayed 2 m-tiles so the DVE never head-of-line
    blocks on the PE).
  * quantize is one merged pass per [128,4096] tile (reduce / 2 scalar
    ops / apply) instead of per-half: fewer DVE instructions.
  * W-phase emits 256-wide strip pieces gated on w-tile PAIRS (N=256
    streams at full PE rate), so matmuls start as soon as w0,w1 + x0 are
    ready and the PE keeps working while W quantization proceeds; the
    first piece is additionally split at K=2048 (PSUM holds the partial
    accumulation).
  * steady state: 512-wide strips, strip-1 delayed two m-tiles.
"""

import numpy as np
import ml_dtypes

import concourse.bass as bass
import concourse.tile as tile
from concourse import bacc, mybir
from concourse._compat import with_exitstack
from concourse.bass_utils import run_bass_kernel_spmd

DT = mybir.dt

M, IN, OUT = 8192, 4096, 4096
PM, PN = 2, 4
M_SH, N_SH = M // PM, OUT // PN  # 4096, 1024 per core
GS = 32          # bfp group size
P = 128          # partitions
NT = 512         # matmul strip width (one PSUM bank of f32)
KC = IN // P     # 32 k-chunks

_C_MUL = 98304.0              # 1.5 * 2^16: C = 2^e * _C_MUL = 1.5*2^23*s
_HI_K = 127.0 / 12582912.0    # C * _HI_K = 127 * s
_LO_K = -1.0 / 98304.0        # C * _LO_K = -128 * s
_EXP_MASK16 = 0x7F80          # bf16 exponent field
_EXP_MIN16 = 0x0080           # clamp exponent field >= 1 (zero-group guard)

# ---------------------------------------------------------------------------
# custom fused DVE op: out = clip(round_to_multiple(x, s), -128s, 127s)
# ---------------------------------------------------------------------------
_BFP_OP = None


def _bfp_apply_ref(in0, in1, c0, c1, c2):
    x = np.asarray(in0, np.float32)
    C = np.asarray(in1, np.float32).reshape(x.shape)
    t = ((x + C).astype(np.float32) - C).astype(np.float32)
    hi = (C * np.float32(c0)).astype(np.float32)
    lo = (C * np.float32(c1)).astype(np.float32)
    return np.maximum(np.minimum(t, hi), lo)


def get_bfp_op():
    global _BFP_OP
    if _BFP_OP is not None:
        return _BFP_OP
    from concourse.dve_ops import (
        CUSTOM_DVE_SPECS,
        OPS,
        _CUSTOM_DVE_ROW_BASE,
        _SUB_OPCODE_FOR_NAME,
        DveOp,
    )
    from concourse.dve_spec import C0, C1, Spec, Src0, Src1, lower, maxx, minn
    from concourse.dve_uop import DveOpSpec

    for existing in OPS:
        if existing.name == "BFP_APPLY_ANT":
            _BFP_OP = existing
            return existing

    t = (Src0 + Src1) - Src1
    spec = Spec(
        body=maxx(minn(t, Src1 * C0), Src1 * C1),
        reference=_bfp_apply_ref,
    )
    shas = {
        ver: DveOpSpec(
            name="BFP_APPLY_ANT", uops=lower(spec, ver=ver), rd1_en=True
        ).sha(ver)
        for ver in ("v3", "v4")
    }
    op = DveOp("BFP_APPLY_ANT", spec, subdim=False, uops_sha=shas)
    OPS.append(op)
    CUSTOM_DVE_SPECS[op.name] = op.spec
    _SUB_OPCODE_FOR_NAME[op.name] = _CUSTOM_DVE_ROW_BASE + len(OPS) - 1
    _BFP_OP = op
    return op


# ---------------------------------------------------------------------------
# Tile kernel builder
# ---------------------------------------------------------------------------
@with_exitstack
def build_bfl(ctx, tc, out_ap, x_ap, w_ap, b_ap, m_sh, n_sh, k):
    nc = tc.nc
    op = get_bfp_op()
    G = k // GS        # groups per row (128)
    kc = k // P        # k-chunks (32)
    n_wt = n_sh // P   # weight row-tiles (8)
    n_mt = m_sh // P   # x row-tiles (32)
    n_nt = n_sh // NT  # 512-wide strips (2)
    wt_per_nt = NT // P  # w tiles per strip (4)
    half = k // 2
    hc = half // P     # chunks per half (16)
    hg = G // 2        # groups per half (64)

    stage = ctx.enter_context(tc.tile_pool(name="stage", bufs=4))
    qpool = ctx.enter_context(tc.tile_pool(name="q", bufs=4))
    qtpool = ctx.enter_context(tc.tile_pool(name="qt", bufs=6))
    gpool = ctx.enter_context(tc.tile_pool(name="g", bufs=3))
    wqt_pool = ctx.enter_context(tc.tile_pool(name="wqt", bufs=1))
    cpool = ctx.enter_context(tc.tile_pool(name="const", bufs=1))
    opool = ctx.enter_context(tc.tile_pool(name="o", bufs=6))
    pspool = ctx.enter_context(tc.tile_pool(name="ps", bufs=5, space="PSUM"))
    psh_pool = ctx.enter_context(tc.tile_pool(name="psh", bufs=3, space="PSUM"))

    # ---- bias broadcast [P, n_sh] via K=1 outer-product matmul ----
    ones_t = cpool.tile([1, P], DT.bfloat16, tag="ones")
    nc.vector.memset(ones_t[:], 1.0)
    bias_f = cpool.tile([1, n_sh], DT.float32, tag="bias_f")
    nc.sync.dma_start(bias_f[:], b_ap.unsqueeze(0))
    bias_r = cpool.tile([1, n_sh], DT.bfloat16, tag="bias_r")
    nc.vector.tensor_copy(bias_r[:], bias_f[:])
    biasb = cpool.tile([P, n_sh], DT.bfloat16, tag="biasb")
    for j in range(n_sh // NT):
        psb = pspool.tile([P, NT], DT.float32, tag="ps512", name=f"psb{j}")
        nc.tensor.matmul(
            psb[:],
            lhsT=ones_t[:],
            rhs=bias_r[:, j * NT:(j + 1) * NT],
            start=True,
            stop=True,
        )
        nc.vector.tensor_copy(biasb[:, j * NT:(j + 1) * NT], psb[:])

    # ---- W quantize: tile wt -> wqt[nt][:, :, col:col+P] ----
    wqt = [
        wqt_pool.tile([P, kc * NT], DT.bfloat16, tag=f"wqt{i}", name=f"wqt{i}")
        for i in range(n_nt)
    ]

    def quantize_tile(src_dram, name, split=False):
        """bf16 rows [P, k] -> bf16 quantized [P, k]; merged 16-bit ops."""
        xb = stage.tile([P, k], DT.bfloat16, tag="stage", name=f"st_{name}")
        nc.scalar.dma_start(xb[:], src_dram)
        gm = gpool.tile([P, G], DT.bfloat16, tag="gmax", name=f"gm_{name}")
        ci = gpool.tile([P, G], DT.int16, tag="ci", name=f"ci_{name}")
        cf = gpool.tile([P, G], DT.bfloat16, tag="cf", name=f"cf_{name}")
        q = qpool.tile([P, k], DT.bfloat16, tag="q", name=f"q_{name}")
        pieces = ((0, half), (half, k))
        for k0, k1 in pieces:
            g0, g1 = k0 // GS, k1 // GS
            nc.vector.tensor_reduce(
                gm[:, g0:g1],
                xb[:, k0:k1].rearrange("p (g j) -> p g j", j=GS),
                axis=mybir.AxisListType.X,
                op=mybir.AluOpType.max,
                apply_absolute_value=True,
            )
            nc.vector.tensor_scalar(
                ci[:, g0:g1],
                gm[:, g0:g1].bitcast(DT.int16),
                _EXP_MASK16,
                None,
                op0=mybir.AluOpType.bitwise_and,
            )
            nc.vector.tensor_scalar_max(
                ci[:, g0:g1], ci[:, g0:g1], _EXP_MIN16
            )
            nc.vector.tensor_scalar_mul(
                cf[:, g0:g1], ci[:, g0:g1].bitcast(DT.bfloat16), _C_MUL
            )
            nc.vector._custom_dve(
                op,
                out=q[:, k0:k1],
                in0=xb[:, k0:k1],
                in1=cf[:, g0:g1].unsqueeze(2).broadcast_to(
                    [P, g1 - g0, GS]
                ),
                s0=_HI_K,
                s1=_LO_K,
            )
        return q

    def w_tile(wt, split=False):
        rows = slice(wt * P, (wt + 1) * P)
        nt, col = wt // wt_per_nt, (wt % wt_per_nt) * P
        w3 = wqt[nt][:].rearrange("p (c n) -> p c n", n=NT)
        q = quantize_tile(w_ap[rows, :], f"w{wt}", split=split)
        for h in range(2):
            nc.sync.dma_start_transpose(
                w3[:, h * hc:(h + 1) * hc, col:col + P],
                q[:, h * half:(h + 1) * half],
            )

    def x_tile(mt, split=False):
        rows = slice(mt * P, (mt + 1) * P)
        xqt = qtpool.tile([P, kc * P], DT.bfloat16, tag="xqt", name=f"xqt{mt}")
        xqt3 = xqt[:].rearrange("p (c m) -> p c m", m=P)
        q = quantize_tile(x_ap[rows, :], f"x{mt}", split=split)
        for h in range(2):
            nc.sync.dma_start_transpose(
                xqt3[:, h * hc:(h + 1) * hc, :],
                q[:, h * half:(h + 1) * half],
            )
        return xqt3

    # ---- strip pieces: accumulating matmuls + (deferred) drain ----
    # drain queue entries: (ps, mt, ncol0, c_w)
    drainq = []

    def flush_drains(keep=0):
        while len(drainq) > keep:
            ps, mt, ncol0, c_w = drainq.pop(0)
            ob = opool.tile([P, c_w], DT.bfloat16, tag=f"o{c_w}",
                            name=f"ob{mt}_{ncol0}")
            nc.scalar.copy(ob[:], ps[:])
            ob2 = opool.tile([P, c_w], DT.bfloat16, tag=f"p{c_w}",
                             name=f"o2{mt}_{ncol0}")
            nc.vector.tensor_tensor(
                ob2[:], ob[:], biasb[:, ncol0:ncol0 + c_w],
                op=mybir.AluOpType.add,
            )
            nc.gpsimd.dma_start(
                out_ap[mt * P:(mt + 1) * P, ncol0:ncol0 + c_w], ob2[:]
            )

    open_ps = {}

    def do_piece(mt, xqt3, c_lo, c_w, k0=0, k1=None, pool=None):
        """cols [c_lo, c_lo+c_w), k-chunks [k0, k1) of m-tile mt."""
        k1 = kc if k1 is None else k1
        key = (mt, c_lo, c_w)
        if key in open_ps:
            ps = open_ps.pop(key)
        else:
            pool = pool or (pspool if c_w == NT else psh_pool)
            ps = pool.tile([P, c_w], DT.float32, tag=f"ps{c_w}",
                           name=f"ps{mt}_{c_lo}")
        nt = c_lo // NT
        s_lo = c_lo - nt * NT
        wq3 = wqt[nt][:].rearrange("p (c n) -> p c n", n=NT)
        for c in range(k0, k1):
            nc.tensor.matmul(
                ps[:],
                lhsT=xqt3[:, c, :],
                rhs=wq3[:, c, s_lo:s_lo + c_w],
                start=(c == 0),
                stop=(c == kc - 1),
            )
        if k1 < kc:
            open_ps[key] = ps
        else:
            drainq.append((ps, mt, c_lo, c_w))

    # ---- emission order ----
    if n_mt < 8 or n_wt != 8 or n_nt != 2:
        # generic order (small shapes / simulator testing)
        for wt in range(n_wt):
            w_tile(wt)
        for mt in range(n_mt):
            xqt3 = x_tile(mt)
            for nt in range(n_nt):
                do_piece(mt, xqt3, nt * NT, NT)
            flush_drains(keep=2)
        flush_drains()
        return

    xq = {}
    # startup: w0,w1 and x0 first; first piece split at K=2048 so the PE
    # starts right after the first half-tiles land.
    w_tile(0)
    w_tile(1)
    xq[0] = x_tile(0)
    do_piece(0, xq[0], 0, 256, k0=0, k1=hc)
    do_piece(0, xq[0], 0, 256, k0=hc)
    w_tile(2)
    w_tile(3)
    do_piece(0, xq[0], 256, 256)
    xq[1] = x_tile(1)
    do_piece(1, xq[1], 0, 256)
    do_piece(1, xq[1], 256, 256)
    w_tile(4)
    w_tile(5)
    xq[2] = x_tile(2)
    do_piece(0, xq[0], 512, 256)
    do_piece(2, xq[2], 0, 256)
    do_piece(1, xq[1], 512, 256)
    do_piece(2, xq[2], 256, 256)
    w_tile(6)
    w_tile(7)
    flush_drains(keep=2)
    xq[3] = x_tile(3)
    do_piece(0, xq[0], 768, 256)
    do_piece(3, xq[3], 0, 512)
    do_piece(1, xq[1], 768, 256)
    do_piece(2, xq[2], 512, 256)
    xq[4] = x_tile(4)
    do_piece(4, xq[4], 0, 512)
    do_piece(2, xq[2], 768, 256)
    do_piece(3, xq[3], 512, 512)
    flush_drains(keep=2)
    # steady state: strip-1 lags strip-0 by one m-tile
    for mt in range(5, n_mt):
        xq[mt] = x_tile(mt)
        do_piece(mt, xq[mt], 0, 512)
        do_piece(mt - 1, xq[mt - 1], 512, 512)
        flush_drains(keep=2)
    do_piece(n_mt - 1, xq[n_mt - 1], 512, 512)
    flush_drains()


# ---------------------------------------------------------------------------
# host entry
# ---------------------------------------------------------------------------
_CACHE = {}
LAST_EXEC_NS = None
LAST_RESULTS = None


def _build(m_sh, n_sh, k, num_devices=8):
    key = (m_sh, n_sh, k)
    if key in _CACHE:
        return _CACHE[key]
    nc = bacc.Bacc(
        "TRN2",
        target_bir_lowering=False,
        debug=False,
        enable_asserts=True,
        num_devices=num_devices,
    )
    x_ap = nc.dram_tensor("x", [m_sh, k], DT.bfloat16, kind="ExternalInput").ap()
    w_ap = nc.dram_tensor("w", [n_sh, k], DT.bfloat16, kind="ExternalInput").ap()
    b_ap = nc.dram_tensor("b", [n_sh], DT.float32, kind="ExternalInput").ap()
    out_ap = nc.dram_tensor(
        "out", [m_sh, n_sh], DT.bfloat16, kind="ExternalOutput"
    ).ap()
    with tile.TileContext(nc) as tc:
        build_bfl(tc, out_ap, x_ap, w_ap, b_ap, m_sh, n_sh, k)
    nc.compile()
    _CACHE[key] = nc
    return nc


def _install_ntff_hook():
    import sys
    import types

    if "antenv.axon_hooks" in sys.modules:
        return
    try:
        from trn_agent_boot.trn_boot import _ntff_profile_via_ctypes

        hook = _ntff_profile_via_ctypes("/opt/axon/libaxon_pjrt.so")
    except Exception:
        hook = None
    mod = types.ModuleType("antenv.axon_hooks")
    state = {"hook": hook}
    mod.get_axon_ntff_profile_hook = lambda: state["hook"]
    mod.set_axon_ntff_profile_hook = lambda h: state.update(hook=h)
    sys.modules["antenv.axon_hooks"] = mod


def kernel(x, weight, bias, trace=False):
    global LAST_EXEC_NS, LAST_RESULTS
    if trace:
        _install_ntff_hook()
    # round-toward-zero f32->bf16 (mantissa truncation): preserves
    # floor(log2(group_max)) exactly, so the BFP exponent matches the
    # f32 reference's (RNE can bump gmax across a power of two, which
    # changes which elements the reference's clip-at-127 slashes).
    x = np.ascontiguousarray(np.asarray(x, np.float32))
    weight = np.ascontiguousarray(np.asarray(weight, np.float32))
    x = (x.view(np.uint32) >> 16).astype(np.uint16).view(ml_dtypes.bfloat16)
    weight = (
        (weight.view(np.uint32) >> 16).astype(np.uint16).view(ml_dtypes.bfloat16)
    )
    bias = np.ascontiguousarray(np.asarray(bias, np.float32))
    assert x.shape == (M, IN) and weight.shape == (OUT, IN) and bias.shape == (OUT,)

    nc = _build(M_SH, N_SH, IN)
    in_maps = []
    for c in range(8):
        mb, nb = c // PN, c % PN
        in_maps.append(
            {
                "x": np.ascontiguousarray(x[mb * M_SH:(mb + 1) * M_SH]),
                "w": np.ascontiguousarray(weight[nb * N_SH:(nb + 1) * N_SH]),
                "b": np.ascontiguousarray(bias[nb * N_SH:(nb + 1) * N_SH]),
            }
        )
    res = run_bass_kernel_spmd(nc, in_maps, core_ids=list(range(8)), trace=trace)
    LAST_EXEC_NS = res.exec_time_ns
    LAST_RESULTS = res
    out = np.empty((M, OUT), np.float32)
    for c in range(8):
        mb, nb = c // PN, c % PN
        out[mb * M_SH:(mb + 1) * M_SH, nb * N_SH:(nb + 1) * N_SH] = np.asarray(
            res.results[c]["out"]
        ).astype(np.float32)
    return out


# revision 22
# speedup vs baseline: 1.1864x; 1.0200x over previous
"""BFP (block-floating-point) quantized linear on 8 TRN2 NeuronCores.

out = quantize_bfp(x) @ quantize_bfp(weight).T + bias
  - groups of 32 contiguous elements along the contraction dim share one
    exponent e = floor(log2(max_abs)); scale s = 2^(e-7);
    q = clip(round(v/s), -128, 127) * s  (round half-to-even), zero-guarded.

Layout: PM=2 x PN=4 grid; each core computes a [4096, 1024] output block
with K=4096, W held resident in SBUF (transposed, bf16), x streamed.

Key design points (727us baseline -> ~610us):
  * host casts x/weight to bf16 with round-toward-zero (mantissa
    truncation): halves load DMA and SBUF staging, and RTZ provably
    preserves floor(log2(group_max)) so the BFP exponent always matches
    the f32 reference (RNE could bump gmax across a power of two and
    change which elements the reference's clip-at-127 slashes).
  * quantize on DVE per half-tile: abs-max group reduce (bf16), exponent
    via int16 bitfield ops, fused round+clip custom DVE op against
    C = 1.5*2^23*s (magic-number rounding, exact in f32 internals).
  * input loads ride the ACT HWDGE queue so the sync queue only carries
    xbar transposes (no head-of-line blocking of prefetches).
  * no bias matmuls on the PE: bias broadcast [128, n_sh] once via a K=1
    outer-product matmul, added during drain on the DVE at 2x; drains
    (ACT copy + DVE add + SWDGE store) trail the matmuls by two strips.
  * startup: first strip piece is 256 wide and split at K=2048 so the PE
    starts as soon as w0,w1 + x0 half-tiles are quantized+transposed;
    W-phase emits 256-wide pieces gated on w-tile pairs.
"""

import numpy as np
import ml_dtypes

import concourse.bass as bass
import concourse.tile as tile
from concourse import bacc, mybir
from concourse._compat import with_exitstack
from concourse.bass_utils import run_bass_kernel_spmd

DT = mybir.dt

M, IN, OUT = 8192, 4096, 4096
PM, PN = 2, 4
M_SH, N_SH = M // PM, OUT // PN  # 4096, 1024 per core
GS = 32          # bfp group size
P = 128          # partitions
NT = 512         # matmul strip width (one PSUM bank of f32)
KC = IN // P     # 32 k-chunks

_C_MUL = 98304.0              # 1.5 * 2^16: C = 2^e * _C_MUL = 1.5*2^23*s
_HI_K = 127.0 / 12582912.0    # C * _HI_K = 127 * s
_LO_K = -1.0 / 98304.0        # C * _LO_K = -128 * s
_EXP_MASK16 = 0x7F80          # bf16 exponent field
_EXP_MIN16 = 0x0080           # clamp exponent field >= 1 (zero-group guard)

# ---------------------------------------------------------------------------
# custom fused DVE op: out = clip(round_to_multiple(x, s), -128s, 127s)
# ---------------------------------------------------------------------------
_BFP_OP = None


def _bfp_apply_ref(in0, in1, c0, c1, c2):
    x = np.asarray(in0, np.float32)
    C = np.asarray(in1, np.float32).reshape(x.shape)
    t = ((x + C).astype(np.float32) - C).astype(np.float32)
    hi = (C * np.float32(c0)).astype(np.float32)
    lo = (C * np.float32(c1)).astype(np.float32)
    return np.maximum(np.minimum(t, hi), lo)


def get_bfp_op():
    global _BFP_OP
    if _BFP_OP is not None:
        return _BFP_OP
    from concourse.dve_ops import (
        CUSTOM_DVE_SPECS,
        OPS,
        _CUSTOM_DVE_ROW_BASE,
        _SUB_OPCODE_FOR_NAME,
        DveOp,
    )
    from concourse.dve_spec import C0, C1, Spec, Src0, Src1, lower, maxx, minn
    from concourse.dve_uop import DveOpSpec

    for existing in OPS:
        if existing.name == "BFP_APPLY_ANT":
            _BFP_OP = existing
            return existing

    t = (Src0 + Src1) - Src1
    spec = Spec(
        body=maxx(minn(t, Src1 * C0), Src1 * C1),
        reference=_bfp_apply_ref,
    )
    shas = {
        ver: DveOpSpec(
            name="BFP_APPLY_ANT", uops=lower(spec, ver=ver), rd1_en=True
        ).sha(ver)
        for ver in ("v3", "v4")
    }
    op = DveOp("BFP_APPLY_ANT", spec, subdim=False, uops_sha=shas)
    OPS.append(op)
    CUSTOM_DVE_SPECS[op.name] = op.spec
    _SUB_OPCODE_FOR_NAME[op.name] = _CUSTOM_DVE_ROW_BASE + len(OPS) - 1
    _BFP_OP = op
    return op


# ---------------------------------------------------------------------------
# Tile kernel builder
# ---------------------------------------------------------------------------
@with_exitstack
def build_bfl(ctx, tc, out_ap, x_ap, w_ap, b_ap, m_sh, n_sh, k):
    nc = tc.nc
    op = get_bfp_op()
    G = k // GS        # groups per row (128)
    kc = k // P        # k-chunks (32)
    n_wt = n_sh // P   # weight row-tiles (8)
    n_mt = m_sh // P   # x row-tiles (32)
    n_nt = n_sh // NT  # 512-wide strips (2)
    wt_per_nt = NT // P  # w tiles per strip (4)
    half = k // 2
    hc = half // P     # chunks per half (16)
    hg = G // 2        # groups per half (64)

    stage = ctx.enter_context(tc.tile_pool(name="stage", bufs=4))
    qpool = ctx.enter_context(tc.tile_pool(name="q", bufs=4))
    qtpool = ctx.enter_context(tc.tile_pool(name="qt", bufs=6))
    gpool = ctx.enter_context(tc.tile_pool(name="g", bufs=3))
    wqt_pool = ctx.enter_context(tc.tile_pool(name="wqt", bufs=1))
    cpool = ctx.enter_context(tc.tile_pool(name="const", bufs=1))
    opool = ctx.enter_context(tc.tile_pool(name="o", bufs=6))
    pspool = ctx.enter_context(tc.tile_pool(name="ps", bufs=5, space="PSUM"))
    psh_pool = ctx.enter_context(tc.tile_pool(name="psh", bufs=3, space="PSUM"))

    # ---- bias broadcast [P, n_sh] via K=1 outer-product matmul ----
    ones_t = cpool.tile([1, P], DT.bfloat16, tag="ones")
    nc.vector.memset(ones_t[:], 1.0)
    bias_f = cpool.tile([1, n_sh], DT.float32, tag="bias_f")
    nc.sync.dma_start(bias_f[:], b_ap.unsqueeze(0))
    bias_r = cpool.tile([1, n_sh], DT.bfloat16, tag="bias_r")
    nc.vector.tensor_copy(bias_r[:], bias_f[:])
    biasb = cpool.tile([P, n_sh], DT.bfloat16, tag="biasb")
    for j in range(n_sh // NT):
        psb = pspool.tile([P, NT], DT.float32, tag="ps512", name=f"psb{j}")
        nc.tensor.matmul(
            psb[:],
            lhsT=ones_t[:],
            rhs=bias_r[:, j * NT:(j + 1) * NT],
            start=True,
            stop=True,
        )
        nc.vector.tensor_copy(biasb[:, j * NT:(j + 1) * NT], psb[:])

    # ---- W quantize: tile wt -> wqt[nt][:, :, col:col+P] ----
    wqt = [
        wqt_pool.tile([P, kc * NT], DT.bfloat16, tag=f"wqt{i}", name=f"wqt{i}")
        for i in range(n_nt)
    ]

    def quantize_tile(src_dram, name, split=False):
        """bf16 rows [P, k] -> bf16 quantized [P, k]; merged 16-bit ops."""
        xb = stage.tile([P, k], DT.bfloat16, tag="stage", name=f"st_{name}")
        nc.scalar.dma_start(xb[:], src_dram)
        gm = gpool.tile([P, G], DT.bfloat16, tag="gmax", name=f"gm_{name}")
        ci = gpool.tile([P, G], DT.int16, tag="ci", name=f"ci_{name}")
        cf = gpool.tile([P, G], DT.bfloat16, tag="cf", name=f"cf_{name}")
        q = qpool.tile([P, k], DT.bfloat16, tag="q", name=f"q_{name}")
        pieces = ((0, half), (half, k))
        for k0, k1 in pieces:
            g0, g1 = k0 // GS, k1 // GS
            nc.vector.tensor_reduce(
                gm[:, g0:g1],
                xb[:, k0:k1].rearrange("p (g j) -> p g j", j=GS),
                axis=mybir.AxisListType.X,
                op=mybir.AluOpType.max,
                apply_absolute_value=True,
            )
            nc.vector.tensor_scalar(
                ci[:, g0:g1],
                gm[:, g0:g1].bitcast(DT.int16),
                _EXP_MASK16,
                None,
                op0=mybir.AluOpType.bitwise_and,
            )
            nc.vector.tensor_scalar_max(
                ci[:, g0:g1], ci[:, g0:g1], _EXP_MIN16
            )
            nc.vector.tensor_scalar_mul(
                cf[:, g0:g1], ci[:, g0:g1].bitcast(DT.bfloat16), _C_MUL
            )
            nc.vector._custom_dve(
                op,
                out=q[:, k0:k1],
                in0=xb[:, k0:k1],
                in1=cf[:, g0:g1].unsqueeze(2).broadcast_to(
                    [P, g1 - g0, GS]
                ),
                s0=_HI_K,
                s1=_LO_K,
            )
        return q

    def w_tile(wt, split=False):
        rows = slice(wt * P, (wt + 1) * P)
        nt, col = wt // wt_per_nt, (wt % wt_per_nt) * P
        w3 = wqt[nt][:].rearrange("p (c n) -> p c n", n=NT)
        q = quantize_tile(w_ap[rows, :], f"w{wt}", split=split)
        for h in range(2):
            nc.sync.dma_start_transpose(
                w3[:, h * hc:(h + 1) * hc, col:col + P],
                q[:, h * half:(h + 1) * half],
            )

    def x_tile(mt, split=False):
        rows = slice(mt * P, (mt + 1) * P)
        xqt = qtpool.tile([P, kc * P], DT.bfloat16, tag="xqt", name=f"xqt{mt}")
        xqt3 = xqt[:].rearrange("p (c m) -> p c m", m=P)
        q = quantize_tile(x_ap[rows, :], f"x{mt}", split=split)
        for h in range(2):
            nc.sync.dma_start_transpose(
                xqt3[:, h * hc:(h + 1) * hc, :],
                q[:, h * half:(h + 1) * half],
            )
        return xqt3

    # ---- strip pieces: accumulating matmuls + (deferred) drain ----
    # drain queue entries: (ps, mt, ncol0, c_w)
    drainq = []

    def flush_drains(keep=0):
        while len(drainq) > keep:
            ps, mt, ncol0, c_w = drainq.pop(0)
            ob = opool.tile([P, c_w], DT.bfloat16, tag=f"o{c_w}",
                            name=f"ob{mt}_{ncol0}")
            nc.scalar.copy(ob[:], ps[:])
            ob2 = opool.tile([P, c_w], DT.bfloat16, tag=f"p{c_w}",
                             name=f"o2{mt}_{ncol0}")
            nc.vector.tensor_tensor(
                ob2[:], ob[:], biasb[:, ncol0:ncol0 + c_w],
                op=mybir.AluOpType.add,
            )
            nc.gpsimd.dma_start(
                out_ap[mt * P:(mt + 1) * P, ncol0:ncol0 + c_w], ob2[:]
            )

    open_ps = {}

    def do_piece(mt, xqt3, c_lo, c_w, k0=0, k1=None, pool=None):
        """cols [c_lo, c_lo+c_w), k-chunks [k0, k1) of m-tile mt."""
        k1 = kc if k1 is None else k1
        key = (mt, c_lo, c_w)
        if key in open_ps:
            ps = open_ps.pop(key)
        else:
            pool = pool or (pspool if c_w == NT else psh_pool)
            ps = pool.tile([P, c_w], DT.float32, tag=f"ps{c_w}",
                           name=f"ps{mt}_{c_lo}")
        nt = c_lo // NT
        s_lo = c_lo - nt * NT
        wq3 = wqt[nt][:].rearrange("p (c n) -> p c n", n=NT)
        for c in range(k0, k1):
            nc.tensor.matmul(
                ps[:],
                lhsT=xqt3[:, c, :],
                rhs=wq3[:, c, s_lo:s_lo + c_w],
                start=(c == 0),
                stop=(c == kc - 1),
            )
        if k1 < kc:
            open_ps[key] = ps
        else:
            drainq.append((ps, mt, c_lo, c_w))

    # ---- emission order ----
    if n_mt < 8 or n_wt != 8 or n_nt != 2:
        # generic order (small shapes / simulator testing)
        for wt in range(n_wt):
            w_tile(wt)
        for mt in range(n_mt):
            xqt3 = x_tile(mt)
            for nt in range(n_nt):
                do_piece(mt, xqt3, nt * NT, NT)
            flush_drains(keep=2)
        flush_drains()
        return

    xq = {}
    # startup: w0,w1 and x0 first; first piece split at K=2048 so the PE
    # starts right after the first half-tiles land.
    w_tile(0)
    w_tile(1)
    xq[0] = x_tile(0)
    do_piece(0, xq[0], 0, 256, k0=0, k1=hc)
    do_piece(0, xq[0], 0, 256, k0=hc)
    w_tile(2)
    w_tile(3)
    do_piece(0, xq[0], 256, 256)
    xq[1] = x_tile(1)
    do_piece(1, xq[1], 0, 256)
    do_piece(1, xq[1], 256, 256)
    w_tile(4)
    w_tile(5)
    xq[2] = x_tile(2)
    do_piece(0, xq[0], 512, 256)
    do_piece(2, xq[2], 0, 256)
    do_piece(1, xq[1], 512, 256)
    do_piece(2, xq[2], 256, 256)
    w_tile(6)
    w_tile(7)
    flush_drains(keep=2)
    xq[3] = x_tile(3)
    do_piece(0, xq[0], 768, 256)
    do_piece(3, xq[3], 0, 512)
    do_piece(1, xq[1], 768, 256)
    do_piece(2, xq[2], 512, 256)
    xq[4] = x_tile(4)
    do_piece(4, xq[4], 0, 512)
    do_piece(2, xq[2], 768, 256)
    do_piece(3, xq[3], 512, 512)
    flush_drains(keep=2)
    # steady state: strip-1 lags strip-0 by one m-tile
    for mt in range(5, n_mt):
        xq[mt] = x_tile(mt)
        do_piece(mt, xq[mt], 0, 512)
        do_piece(mt - 1, xq[mt - 1], 512, 512)
        flush_drains(keep=2)
    do_piece(n_mt - 1, xq[n_mt - 1], 512, 512)
    flush_drains()


# ---------------------------------------------------------------------------
# host entry
# ---------------------------------------------------------------------------
_CACHE = {}
LAST_EXEC_NS = None
LAST_RESULTS = None


def _build(m_sh, n_sh, k, num_devices=8):
    key = (m_sh, n_sh, k)
    if key in _CACHE:
        return _CACHE[key]
    nc = bacc.Bacc(
        "TRN2",
        target_bir_lowering=False,
        debug=False,
        enable_asserts=True,
        num_devices=num_devices,
    )
    x_ap = nc.dram_tensor("x", [m_sh, k], DT.bfloat16, kind="ExternalInput").ap()
    w_ap = nc.dram_tensor("w", [n_sh, k], DT.bfloat16, kind="ExternalInput").ap()
    b_ap = nc.dram_tensor("b", [n_sh], DT.float32, kind="ExternalInput").ap()
    out_ap = nc.dram_tensor(
        "out", [m_sh, n_sh], DT.bfloat16, kind="ExternalOutput"
    ).ap()
    with tile.TileContext(nc) as tc:
        build_bfl(tc, out_ap, x_ap, w_ap, b_ap, m_sh, n_sh, k)
    nc.compile()
    _CACHE[key] = nc
    return nc


def _install_ntff_hook():
    import sys
    import types

    if "antenv.axon_hooks" in sys.modules:
        return
    try:
        from trn_agent_boot.trn_boot import _ntff_profile_via_ctypes

        hook = _ntff_profile_via_ctypes("/opt/axon/libaxon_pjrt.so")
    except Exception:
        hook = None
    mod = types.ModuleType("antenv.axon_hooks")
    state = {"hook": hook}
    mod.get_axon_ntff_profile_hook = lambda: state["hook"]
    mod.set_axon_ntff_profile_hook = lambda h: state.update(hook=h)
    sys.modules["antenv.axon_hooks"] = mod


def kernel(x, weight, bias, trace=False):
    global LAST_EXEC_NS, LAST_RESULTS
    if trace:
        _install_ntff_hook()
    # round-toward-zero f32->bf16 (mantissa truncation): preserves
    # floor(log2(group_max)) exactly, so the BFP exponent matches the
    # f32 reference's (RNE can bump gmax across a power of two, which
    # changes which elements the reference's clip-at-127 slashes).
    x = np.ascontiguousarray(np.asarray(x, np.float32))
    weight = np.ascontiguousarray(np.asarray(weight, np.float32))
    x = (x.view(np.uint32) >> 16).astype(np.uint16).view(ml_dtypes.bfloat16)
    weight = (
        (weight.view(np.uint32) >> 16).astype(np.uint16).view(ml_dtypes.bfloat16)
    )
    bias = np.ascontiguousarray(np.asarray(bias, np.float32))
    assert x.shape == (M, IN) and weight.shape == (OUT, IN) and bias.shape == (OUT,)

    nc = _build(M_SH, N_SH, IN)
    in_maps = []
    for c in range(8):
        mb, nb = c // PN, c % PN
        in_maps.append(
            {
                "x": np.ascontiguousarray(x[mb * M_SH:(mb + 1) * M_SH]),
                "w": np.ascontiguousarray(weight[nb * N_SH:(nb + 1) * N_SH]),
                "b": np.ascontiguousarray(bias[nb * N_SH:(nb + 1) * N_SH]),
            }
        )
    res = run_bass_kernel_spmd(nc, in_maps, core_ids=list(range(8)), trace=trace)
    LAST_EXEC_NS = res.exec_time_ns
    LAST_RESULTS = res
    out = np.empty((M, OUT), np.float32)
    for c in range(8):
        mb, nb = c // PN, c % PN
        out[mb * M_SH:(mb + 1) * M_SH, nb * N_SH:(nb + 1) * N_SH] = np.asarray(
            res.results[c]["out"]
        ).astype(np.float32)
    return out
